# revision 6
# baseline (speedup 1.0000x reference)
"""DPPConv2d Trainium2 Bass kernel (bf16 pipeline).

Reference computation (per sample s):
  pooled = mean_{h,w} x[s]                              [Cin]
  h      = relu(pooled @ W1.T)                          [hidden]
  logits = h @ W2.T + b2                                [P*Cout]
  attn   = softmax(logits.reshape(P, Cout) / 0.5, p)    [P, Cout]
  m      = (mean_{o,i}(|W[p,:,:,k,l]| - thr[p,:]) > 0)  [P, K, K]
  agg    = sum_p attn[p, co] * m[p, kl] * W[p, co, ci, kl]
  out[s] = conv2d(x[s], agg, pad=1)                     [Cout, H, W]

Sharding: data-parallel over batch -- 8 cores x 4 samples each; the
weight bank / psa weights / threshold are replicated on every core.

Key performance facts (HW-measured):
  - bf16 matmul streams 2 moving columns/cycle (FD=512 -> ~111 ns) and
    its LDWEIGHTS is hidden by the PE reorder window; the f32r
    self-loading encoding pays the weight load inline (~178 ns).
  - So everything is bf16: x is padded+converted on the host, the
    weight bank is host-reordered to [P, Cout, k*k, Cin] bf16, conv
    accumulates in fp32 PSUM, output returns as bf16.

Per-core device pipeline:
  - SE attention entirely in [Cout, P, S] layout (free dims of 4..16,
    using stride-0 broadcast APs) -- logits via 4 tiny matmuls with
    host-pretransposed W2; no attention transposes needed.
  - binary spatial mask on device (|W| reduce, ones-matmul partition
    sums, is_gt, ones-matmul broadcast), folded into the bank (wm).
  - per-sample aggregated weights via tensor_scalar/stt chains in
    [co, kl*ci] layout with attn as per-partition scalar.
  - lhsT [ci, kl, co] built with 9 SBUF->SBUF DMA transposes (XBAR),
    zero PE/scalar cost.
  - conv as 9 shifted accumulating matmuls per 8-row output chunk.
"""

import os
import sys

try:
    import concourse.bass as bass  # noqa: F401
except Exception:  # pragma: no cover
    sys.path.insert(0, "/opt/trn_rl_repo")

from contextlib import ExitStack

import numpy as np
import ml_dtypes

import concourse.bass as bass
import concourse.tile as tile
from concourse import mybir
from concourse.bass_utils import run_bass_kernel_spmd

N_CORES = 8
BS = 32
BS_LOCAL = BS // N_CORES  # 4
CIN = 128
COUT = 128
H = W = 64
P_PAT = 4
KS = 3
KK = KS * KS
HID = 33
TEMP = 0.5
YC = 8          # output rows per conv chunk
N_CHUNK = H // YC

F32 = mybir.dt.float32
BF16 = mybir.dt.bfloat16


def build_nc(rep=1, tp_mode=None):
    if tp_mode is None:
        tp_mode = os.environ.get("DPP_TP", "dma")
    nc = bass.Bass("TRN2", target_bir_lowering=False, debug=False,
                   num_swdge_queues=4)

    x_d = nc.dram_tensor("x", [BS_LOCAL, CIN, H + 2, W + 2], BF16,
                         kind="ExternalInput")
    idb_d = nc.dram_tensor("identb", [128, 128], BF16, kind="ExternalInput")
    wb_d = nc.dram_tensor("wb", [P_PAT, COUT, KK, CIN], BF16,
                          kind="ExternalInput")
    w1_d = nc.dram_tensor("w1T", [CIN, HID], F32, kind="ExternalInput")
    w2_d = nc.dram_tensor("w2T", [HID, P_PAT, COUT], F32, kind="ExternalInput")
    b2_d = nc.dram_tensor("b2cp", [COUT, P_PAT], F32, kind="ExternalInput")
    th_d = nc.dram_tensor("thrm", [1, P_PAT], F32, kind="ExternalInput")
    out_d = nc.dram_tensor("out", [BS_LOCAL, COUT, H, W], BF16,
                           kind="ExternalOutput")

    with tile.TileContext(nc) as tc, ExitStack() as ctx:
        consts = ctx.enter_context(tc.tile_pool(name="consts", bufs=1))
        spool = ctx.enter_context(tc.tile_pool(name="spool", bufs=2))
        xpool = ctx.enter_context(tc.tile_pool(name="xpool", bufs=2))
        wpool = ctx.enter_context(tc.tile_pool(name="wpool", bufs=2))
        aggp = ctx.enter_context(tc.tile_pool(name="aggp", bufs=2))
        lhsp = ctx.enter_context(tc.tile_pool(name="lhsp", bufs=3))
        outp = ctx.enter_context(tc.tile_pool(name="outp", bufs=2))
        ps_small = ctx.enter_context(
            tc.tile_pool(name="ps_small", bufs=2, space="PSUM"))
        ps_mm = ctx.enter_context(
            tc.tile_pool(name="ps_mm", bufs=4, space="PSUM"))
        if tp_mode == "pe":
            ps_tp = ctx.enter_context(
                tc.tile_pool(name="ps_tp", bufs=2, space="PSUM"))

        ones_col = consts.tile([128, 1], F32, tag="ones_col")
        nc.vector.memset(ones_col[:], 1.0)
        ones_row = consts.tile([1, 128], F32, tag="ones_row")
        nc.vector.memset(ones_row[:], 1.0)
        idb = consts.tile([128, 128], BF16, tag="idb")
        nc.sync.dma_start(idb[:], idb_d[:])

        for _rep in range(rep):
            # ---- input DMAs ---------------------------------------------
            xs = [xpool.tile([CIN, H + 2, W + 2], BF16, tag=f"xs{s}",
                             name=f"xs{s}") for s in range(BS_LOCAL)]
            for s in range(BS_LOCAL):
                nc.gpsimd.dma_start(xs[s][:], x_d[s])
            wco = wpool.tile([128, P_PAT, KK, CIN], BF16, tag="wco")
            nc.sync.dma_start(
                wco[:], wb_d[:].rearrange("p co kl ci -> co p kl ci"))
            w1sb = spool.tile([CIN, HID], F32, tag="w1sb")
            nc.sync.dma_start(w1sb[:], w1_d[:])
            w2sb = spool.tile([HID, P_PAT, COUT], F32, tag="w2sb")
            nc.sync.dma_start(w2sb[:], w2_d[:])
            b2sb = spool.tile([COUT, P_PAT], F32, tag="b2sb")
            nc.sync.dma_start(b2sb[:], b2_d[:])
            thsb = spool.tile([1, P_PAT], F32, tag="thsb")
            nc.sync.dma_start(thsb[:], th_d[:])

            # ---- pooled sums (mean folded into the relu scale) ----------
            pooled = spool.tile([CIN, BS_LOCAL], F32, tag="pooled")
            for s in range(BS_LOCAL):
                nc.vector.reduce_sum(
                    pooled[:, s:s + 1], xs[s][:], axis=mybir.AxisListType.XY)

            # ---- binary spatial mask ------------------------------------
            wsum = spool.tile([128, P_PAT * KK], F32, tag="wsum")
            nc.vector.reduce_sum(
                wsum[:].rearrange("co (p kl) -> co p kl", p=P_PAT),
                wco[:], axis=mybir.AxisListType.X,
                apply_absolute_value=True)
            ws_ps = ps_small.tile([1, P_PAT * KK], F32, tag="sm")
            nc.tensor.matmul(ws_ps[:], ones_col[:], wsum[:])
            z_row = spool.tile([1, P_PAT * KK], F32, tag="z_row")
            for p in range(P_PAT):
                nc.vector.tensor_scalar(
                    z_row[0:1, p * KK:(p + 1) * KK],
                    ws_ps[0:1, p * KK:(p + 1) * KK],
                    1.0 / (COUT * CIN), thsb[0:1, p:p + 1],
                    op0=mybir.AluOpType.mult, op1=mybir.AluOpType.subtract)
            m_row = spool.tile([1, P_PAT * KK], F32, tag="m_row")
            nc.vector.tensor_scalar(
                m_row[:], z_row[:], 0.0, None, op0=mybir.AluOpType.is_gt)
            mb_ps = ps_small.tile([128, P_PAT * KK], F32, tag="sm")
            nc.tensor.matmul(mb_ps[:], ones_row[:], m_row[:])
            mb = spool.tile([128, P_PAT * KK], F32, tag="mb")
            nc.scalar.copy(mb[:], mb_ps[:])

            # ---- SE attention MLP in [co, p, s] layout ------------------
            h_ps = ps_small.tile([HID, BS_LOCAL], F32, tag="sm")
            nc.tensor.matmul(h_ps[:], w1sb[:], pooled[:])
            h_sb = spool.tile([HID, BS_LOCAL], F32, tag="h_sb")
            nc.scalar.activation(
                h_sb[:], h_ps[:], mybir.ActivationFunctionType.Relu,
                scale=1.0 / (H * W))
            lg_ps = ps_small.tile([COUT, P_PAT, BS_LOCAL], F32, tag="sm")
            for p in range(P_PAT):
                nc.tensor.matmul(lg_ps[:, p], w2sb[:, p], h_sb[:])
            lgb = spool.tile([COUT, P_PAT, BS_LOCAL], F32, tag="lgb")
            nc.vector.tensor_add(
                lgb[:], lg_ps[:],
                b2sb[:].unsqueeze(2).broadcast_to((COUT, P_PAT, BS_LOCAL)))

            # softmax over the pattern axis (temperature 0.5 -> scale 2.0)
            mx = spool.tile([COUT, BS_LOCAL], F32, tag="mx")
            nc.vector.tensor_max(mx[:], lgb[:, 0], lgb[:, 1])
            nc.vector.tensor_max(mx[:], mx[:], lgb[:, 2])
            nc.vector.tensor_max(mx[:], mx[:], lgb[:, 3])
            sd = spool.tile([COUT, P_PAT, BS_LOCAL], F32, tag="sd")
            nc.vector.tensor_sub(
                sd[:], lgb[:],
                mx[:].unsqueeze(1).broadcast_to((COUT, P_PAT, BS_LOCAL)))
            se = spool.tile([COUT, P_PAT, BS_LOCAL], F32, tag="se")
            nc.scalar.activation(
                se[:], sd[:], mybir.ActivationFunctionType.Exp,
                scale=1.0 / TEMP)
            ssum = spool.tile([COUT, BS_LOCAL], F32, tag="ssum")
            nc.vector.tensor_add(ssum[:], se[:, 0], se[:, 1])
            nc.vector.tensor_add(ssum[:], ssum[:], se[:, 2])
            nc.vector.tensor_add(ssum[:], ssum[:], se[:, 3])
            rec = spool.tile([COUT, BS_LOCAL], F32, tag="rec")
            nc.vector.reciprocal(rec[:], ssum[:])
            attn = spool.tile([COUT, P_PAT, BS_LOCAL], F32, tag="attn")
            nc.vector.tensor_mul(
                attn[:], se[:],
                rec[:].unsqueeze(1).broadcast_to((COUT, P_PAT, BS_LOCAL)))

            # ---- fold mask into the weight bank -------------------------
            wm = spool.tile([128, P_PAT, KK, CIN], BF16, tag="wm")
            for p in range(P_PAT):
                for kl in range(KK):
                    nc.vector.tensor_scalar_mul(
                        wm[:, p, kl], wco[:, p, kl],
                        mb[:, p * KK + kl:p * KK + kl + 1])

            # ---- per-sample: aggregate, transpose, convolve -------------
            for s in range(BS_LOCAL):
                agg = aggp.tile([128, KK, CIN], BF16, tag="agg")
                av = agg[:].rearrange("co kl ci -> co (kl ci)")
                nc.vector.tensor_scalar_mul(
                    av, wm[:, 0].rearrange("co kl ci -> co (kl ci)"),
                    attn[:, 0, s:s + 1])
                for p in range(1, P_PAT):
                    nc.vector.scalar_tensor_tensor(
                        av, wm[:, p].rearrange("co kl ci -> co (kl ci)"),
                        attn[:, p, s:s + 1], av,
                        op0=mybir.AluOpType.mult, op1=mybir.AluOpType.add)

                lhsT = lhsp.tile([CIN, KK, COUT], BF16, tag="lhsT")
                if tp_mode == "dma":
                    for kl in range(KK):
                        eng = nc.sync if kl % 2 == 0 else nc.scalar
                        eng.dma_start_transpose(lhsT[:, kl], agg[:, kl])
                else:
                    for kl in range(KK):
                        tp_ps = ps_tp.tile([CIN, COUT], BF16, tag="tp")
                        nc.tensor.transpose(tp_ps[:], agg[:, kl], idb[:])
                        nc.scalar.copy(lhsT[:, kl], tp_ps[:])

                osb = outp.tile([COUT, H, W], BF16, tag="osb")
                for yc in range(N_CHUNK):
                    y0 = yc * YC
                    pt = ps_mm.tile([COUT, YC, W], F32, tag="pt")
                    for i, (dk, dl) in enumerate(
                            (dk, dl) for dk in range(KS) for dl in range(KS)):
                        nc.tensor.matmul(
                            pt[:],
                            lhsT[:, dk * KS + dl],
                            xs[s][:, y0 + dk:y0 + dk + YC, dl:dl + W],
                            start=(i == 0), stop=(i == KK - 1))
                    nc.scalar.copy(osb[:, y0:y0 + YC, :], pt[:])
                nc.sync.dma_start(out_d[s], osb[:])

    _split_excess_waits(nc)
    return nc


def _split_excess_waits(nc, max_inline=1):
    """Hoist extra sync waits into standalone EventSemaphore instructions.

    This walrus build rejects instructions whose encoded sync-command
    count exceeds the ISA struct capacity ("Too many sync wait
    commands") -- in practice more than one wait per compute
    instruction. Engines execute their instruction stream in order, so
    blocking on a preceding same-engine EventSemaphore is equivalent to
    the instruction carrying the wait itself.
    """
    n = 0
    for f in nc.m.functions:
        for blk in f.blocks:
            out = []
            for inst in blk.instructions:
                si = inst.sync_info
                if si is not None and len(si.on_wait) > max_inline:
                    waits = list(si.on_wait)
                    keep = waits[:max_inline]
                    for w in waits[max_inline:]:
                        n += 1
                        ev = mybir.InstEventSemaphore(
                            name=f"WSPLIT-{n}", ins=[], outs=[])
                        ev.engine = inst.engine
                        ev.sync_info = mybir.SyncInfo(on_wait=[w], on_update=[])
                        ev.debug = inst.debug
                        nc.inst_map[ev.name] = ev
                        out.append(ev)
                    inst.sync_info = mybir.SyncInfo(
                        on_wait=keep, on_update=list(si.on_update))
                out.append(inst)
            blk.instructions = out
    return n


class _Runner:
    """Cached PJRT executor for the 8-core SPMD program.

    Mirrors bass2jax.run_bass_via_pjrt's multi-core path but keeps the
    jitted shard_map callable (and the device mesh) alive across calls,
    so repeat invocations skip retracing and recompilation.
    """

    def __init__(self, nc):
        import jax
        import jax.numpy as jnp
        from jax.experimental.shard_map import shard_map
        from jax.sharding import Mesh, PartitionSpec, NamedSharding
        from concourse import bass2jax, mybir as _mb

        bass2jax.install_neuronx_cc_hook()
        self.jax = jax
        self.nc = nc
        assert nc.dbg_addr is None

        partition_name = (nc.partition_id_tensor.name
                          if nc.partition_id_tensor else None)
        in_names, out_names, out_avals, zero_shapes = [], [], [], []
        for alloc in nc.m.functions[0].allocations:
            if not isinstance(alloc, _mb.MemoryLocationSet):
                continue
            name = alloc.memorylocations[0].name
            if alloc.kind == "ExternalInput":
                if name != partition_name:
                    in_names.append(name)
            elif alloc.kind == "ExternalOutput":
                out_names.append(name)
                shape = tuple(alloc.tensor_shape)
                dtype = _mb.dt.np(alloc.dtype)
                out_avals.append(jax.core.ShapedArray(shape, dtype))
                zero_shapes.append((shape, dtype))
        self.in_names = list(in_names)
        self.out_names = out_names
        self.out_avals = out_avals
        n_params = len(in_names)
        n_outs = len(out_names)
        all_in_names = in_names + out_names
        if partition_name is not None:
            all_in_names.append(partition_name)
        donate = tuple(range(n_params, n_params + n_outs))

        def _body(*args):
            operands = list(args)
            if partition_name is not None:
                operands.append(bass2jax.partition_id_tensor())
            outs = bass2jax._bass_exec_p.bind(
                *operands,
                out_avals=tuple(out_avals),
                in_names=tuple(all_in_names),
                out_names=tuple(out_names),
                lowering_input_output_aliases=(),
                sim_require_finite=True,
                sim_require_nnan=True,
                nc=nc,
            )
            return tuple(outs)

        devices = jax.devices()[:N_CORES]
        self.mesh = Mesh(np.asarray(devices), ("core",))
        self.sharding = NamedSharding(self.mesh, PartitionSpec("core"))
        in_specs = (PartitionSpec("core"),) * (n_params + n_outs)
        out_specs = (PartitionSpec("core"),) * n_outs
        self.sharded = jax.jit(
            shard_map(_body, mesh=self.mesh, in_specs=in_specs,
                      out_specs=out_specs, check_rep=False),
            donate_argnums=donate, keep_unused=True)
        self._zero_makers = [
            jax.jit(
                (lambda sh=sh, dt=dt: jnp.zeros((N_CORES * sh[0],) + sh[1:], dt)),
                out_shardings=self.sharding)
            for sh, dt in zero_shapes
        ]

    def put_inputs(self, in_maps):
        """Concat per-core inputs on axis 0 and upload sharded."""
        cat = [
            np.concatenate([np.asarray(m[name]) for m in in_maps], axis=0)
            for name in self.in_names
        ]
        return [self.jax.device_put(a, self.sharding) for a in cat]

    def run(self, dev_inputs):
        zeros = [zm() for zm in self._zero_makers]
        outs = self.sharded(*dev_inputs, *zeros)
        self.jax.block_until_ready(outs)
        return outs

    def results(self, outs):
        res = []
        for c in range(N_CORES):
            res.append({
                name: np.asarray(outs[i]).reshape(
                    N_CORES, *self.out_avals[i].shape)[c]
                for i, name in enumerate(self.out_names)
            })
        return res


_RUNNER_CACHE = {}


def _get_runner(use_f32r=True, rep=1, mm=None):
    key = rep
    if key not in _RUNNER_CACHE:
        _RUNNER_CACHE[key] = _Runner(_get_nc(rep=rep))
    return _RUNNER_CACHE[key]


_NC_CACHE = {}


def _get_nc(use_f32r=True, rep=1, mm=None):
    key = rep
    if key not in _NC_CACHE:
        _NC_CACHE[key] = build_nc(rep=rep)
    return _NC_CACHE[key]


def make_in_maps(x, psa_w1, psa_w2, psa_b2, weight, threshold):
    x = np.asarray(x, dtype=np.float32)
    xp = np.zeros((BS, CIN, H + 2, W + 2), np.float32)
    xp[:, :, 1:H + 1, 1:W + 1] = x
    xp = xp.astype(ml_dtypes.bfloat16)
    w = np.asarray(weight, np.float32)
    wb = np.ascontiguousarray(
        w.reshape(P_PAT, COUT, CIN, KK).transpose(0, 1, 3, 2)
    ).astype(ml_dtypes.bfloat16)
    w2 = np.asarray(psa_w2, np.float32).reshape(P_PAT, COUT, HID)
    thr = np.asarray(threshold, np.float32).reshape(P_PAT, COUT)
    common = {
        "identb": np.eye(128, dtype=ml_dtypes.bfloat16),
        "wb": wb,
        "w1T": np.ascontiguousarray(np.asarray(psa_w1, np.float32).T),
        "w2T": np.ascontiguousarray(w2.transpose(2, 0, 1)),
        "b2cp": np.ascontiguousarray(
            np.asarray(psa_b2, np.float32).reshape(P_PAT, COUT).T),
        "thrm": thr.mean(axis=1, keepdims=True).T,  # [1, P]
    }
    return [
        {"x": xp[c * BS_LOCAL:(c + 1) * BS_LOCAL], **common}
        for c in range(N_CORES)
    ]


def kernel(x, psa_w1, psa_w2, psa_b2, weight, threshold):
    in_maps = make_in_maps(x, psa_w1, psa_w2, psa_b2, weight, threshold)
    try:
        r = _get_runner()
        outs = r.run(r.put_inputs(in_maps))
        res = r.results(outs)
    except Exception:
        nc = _get_nc()
        res = run_bass_kernel_spmd(nc, in_maps, list(range(N_CORES))).results
    return np.concatenate(
        [res[c]["out"] for c in range(N_CORES)], axis=0).astype(np.float32)


# revision 12
# speedup vs baseline: 21.9979x; 21.9979x over previous
"""DPPConv2d Trainium2 Bass kernel (bf16 pipeline).

Reference computation (per sample s):
  pooled = mean_{h,w} x[s]                              [Cin]
  h      = relu(pooled @ W1.T)                          [hidden]
  logits = h @ W2.T + b2                                [P*Cout]
  attn   = softmax(logits.reshape(P, Cout) / 0.5, p)    [P, Cout]
  m      = (mean_{o,i}(|W[p,:,:,k,l]| - thr[p,:]) > 0)  [P, K, K]
  agg    = sum_p attn[p, co] * m[p, kl] * W[p, co, ci, kl]
  out[s] = conv2d(x[s], agg, pad=1)                     [Cout, H, W]

Sharding: data-parallel over batch -- 8 cores x 4 samples each; the
weight bank / psa weights / threshold are replicated on every core.

Key performance facts (HW-measured):
  - bf16 matmul streams 2 moving columns/cycle (FD=512 -> ~111 ns) and
    its LDWEIGHTS is hidden by the PE reorder window; the f32r
    self-loading encoding pays the weight load inline (~178 ns).
  - So everything is bf16: x is padded+converted on the host, the
    weight bank is host-reordered to [P, Cout, k*k, Cin] bf16, conv
    accumulates in fp32 PSUM, output returns as bf16.

Per-core device pipeline:
  - SE attention entirely in [Cout, P, S] layout (free dims of 4..16,
    using stride-0 broadcast APs) -- logits via 4 tiny matmuls with
    host-pretransposed W2; no attention transposes needed.
  - binary spatial mask on device (|W| reduce, ones-matmul partition
    sums, is_gt, ones-matmul broadcast), folded into the bank (wm).
  - per-sample aggregated weights via tensor_scalar/stt chains in
    [co, kl*ci] layout with attn as per-partition scalar.
  - lhsT [ci, kl, co] built with 9 SBUF->SBUF DMA transposes (XBAR),
    zero PE/scalar cost.
  - conv as 9 shifted accumulating matmuls per 8-row output chunk.
"""

import os
import sys

try:
    import concourse.bass as bass  # noqa: F401
except Exception:  # pragma: no cover
    sys.path.insert(0, "/opt/trn_rl_repo")

from contextlib import ExitStack

import numpy as np
import ml_dtypes

import concourse.bass as bass
import concourse.tile as tile
from concourse import mybir
from concourse.bass_utils import run_bass_kernel_spmd

N_CORES = 8
BS = 32
BS_LOCAL = BS // N_CORES  # 4
CIN = 128
COUT = 128
H = W = 64
P_PAT = 4
KS = 3
KK = KS * KS
HID = 33
TEMP = 0.5
YC = 8          # output rows per conv chunk
N_CHUNK = H // YC

F32 = mybir.dt.float32
BF16 = mybir.dt.bfloat16


def build_nc(rep=1, tp_mode=None):
    if tp_mode is None:
        tp_mode = os.environ.get("DPP_TP", "dma")
    cut = set(os.environ.get("DPP_CUT", "").split(","))  # timing bisect only
    nc = bass.Bass("TRN2", target_bir_lowering=False, debug=False,
                   num_swdge_queues=4)

    x_d = nc.dram_tensor("x", [BS_LOCAL, CIN, H + 2, W + 2], BF16,
                         kind="ExternalInput")
    idb_d = nc.dram_tensor("identb", [128, 128], BF16, kind="ExternalInput")
    wb_d = nc.dram_tensor("wb", [P_PAT, COUT, KK, CIN], BF16,
                          kind="ExternalInput")
    w1_d = nc.dram_tensor("w1T", [CIN, HID], F32, kind="ExternalInput")
    w2_d = nc.dram_tensor("w2T", [HID, P_PAT, COUT], F32, kind="ExternalInput")
    b2_d = nc.dram_tensor("b2cp", [COUT, P_PAT], F32, kind="ExternalInput")
    th_d = nc.dram_tensor("thrm", [1, P_PAT], F32, kind="ExternalInput")
    out_d = nc.dram_tensor("out", [BS_LOCAL, COUT, H, W], BF16,
                           kind="ExternalOutput")

    with tile.TileContext(nc) as tc, ExitStack() as ctx:
        consts = ctx.enter_context(tc.tile_pool(name="consts", bufs=1))
        spool = ctx.enter_context(tc.tile_pool(name="spool", bufs=2))
        xpool = ctx.enter_context(tc.tile_pool(name="xpool", bufs=2))
        wpool = ctx.enter_context(tc.tile_pool(name="wpool", bufs=2))
        aggp = ctx.enter_context(tc.tile_pool(name="aggp", bufs=2))
        lhsp = ctx.enter_context(tc.tile_pool(name="lhsp", bufs=3))
        outp = ctx.enter_context(tc.tile_pool(name="outp", bufs=2))
        ps_small = ctx.enter_context(
            tc.tile_pool(name="ps_small", bufs=2, space="PSUM"))
        ps_mm = ctx.enter_context(
            tc.tile_pool(name="ps_mm", bufs=4, space="PSUM"))
        if tp_mode == "pe":
            ps_tp = ctx.enter_context(
                tc.tile_pool(name="ps_tp", bufs=2, space="PSUM"))

        ones_col = consts.tile([128, 1], F32, tag="ones_col")
        nc.vector.memset(ones_col[:], 1.0)
        ones_row = consts.tile([1, 128], F32, tag="ones_row")
        nc.vector.memset(ones_row[:], 1.0)
        idb = consts.tile([128, 128], BF16, tag="idb")
        nc.sync.dma_start(idb[:], idb_d[:])
        if "agg" in cut:
            lhsT_c = consts.tile([CIN, KK, COUT], BF16, tag="lhsT_c")
            nc.vector.memset(lhsT_c[:].bitcast(F32), 0.5)
        if "se" in cut:
            attn_c = consts.tile([COUT, P_PAT, BS_LOCAL], F32, tag="attn_c")
            nc.vector.memset(attn_c[:], 0.25)

        for _rep in range(rep):
            # ---- input DMAs ---------------------------------------------
            xs = [xpool.tile([CIN, H + 2, W + 2], BF16, tag=f"xs{s}",
                             name=f"xs{s}") for s in range(BS_LOCAL)]
            for s in range(BS_LOCAL):
                nc.gpsimd.dma_start(xs[s][:], x_d[s])
            wco = wpool.tile([128, P_PAT, KK, CIN], BF16, tag="wco")
            nc.sync.dma_start(
                wco[:], wb_d[:].rearrange("p co kl ci -> co p kl ci"))
            w1sb = spool.tile([CIN, HID], F32, tag="w1sb")
            nc.sync.dma_start(w1sb[:], w1_d[:])
            w2sb = spool.tile([HID, P_PAT, COUT], F32, tag="w2sb")
            nc.sync.dma_start(w2sb[:], w2_d[:])
            b2sb = spool.tile([COUT, P_PAT], F32, tag="b2sb")
            nc.sync.dma_start(b2sb[:], b2_d[:])
            thsb = spool.tile([1, P_PAT], F32, tag="thsb")
            nc.sync.dma_start(thsb[:], th_d[:])

            # ---- pooled sums (mean folded into the relu scale) ----------
            pooled = spool.tile([CIN, BS_LOCAL], F32, tag="pooled")
            if "se" not in cut:
                for s in range(BS_LOCAL):
                    nc.vector.reduce_sum(
                        pooled[:, s:s + 1], xs[s][:],
                        axis=mybir.AxisListType.XY)

            # ---- binary spatial mask ------------------------------------
            wsum = spool.tile([128, P_PAT * KK], F32, tag="wsum")
            if "mask" in cut:
                wm = wco
            else:
                nc.vector.reduce_sum(
                    wsum[:].rearrange("co (p kl) -> co p kl", p=P_PAT),
                    wco[:], axis=mybir.AxisListType.X,
                    apply_absolute_value=True)
                ws_ps = ps_small.tile([1, P_PAT * KK], F32, tag="sm")
                nc.tensor.matmul(ws_ps[:], ones_col[:], wsum[:])
                z_row = spool.tile([1, P_PAT * KK], F32, tag="z_row")
                for p in range(P_PAT):
                    nc.vector.tensor_scalar(
                        z_row[0:1, p * KK:(p + 1) * KK],
                        ws_ps[0:1, p * KK:(p + 1) * KK],
                        1.0 / (COUT * CIN), thsb[0:1, p:p + 1],
                        op0=mybir.AluOpType.mult,
                        op1=mybir.AluOpType.subtract)
                m_row = spool.tile([1, P_PAT * KK], F32, tag="m_row")
                nc.vector.tensor_scalar(
                    m_row[:], z_row[:], 0.0, None, op0=mybir.AluOpType.is_gt)
                mb_ps = ps_small.tile([128, P_PAT * KK], F32, tag="sm")
                nc.tensor.matmul(mb_ps[:], ones_row[:], m_row[:])
                mb = spool.tile([128, P_PAT * KK], F32, tag="mb")
                nc.scalar.copy(mb[:], mb_ps[:])

            # ---- SE attention MLP in [co, p, s] layout ------------------
            if "se" in cut:
                attn = attn_c
            else:
                h_ps = ps_small.tile([HID, BS_LOCAL], F32, tag="sm")
                nc.tensor.matmul(h_ps[:], w1sb[:], pooled[:])
                h_sb = spool.tile([HID, BS_LOCAL], F32, tag="h_sb")
                nc.scalar.activation(
                    h_sb[:], h_ps[:], mybir.ActivationFunctionType.Relu,
                    scale=1.0 / (H * W))
                lg_ps = ps_small.tile([COUT, P_PAT, BS_LOCAL], F32, tag="sm")
                for p in range(P_PAT):
                    nc.tensor.matmul(lg_ps[:, p], w2sb[:, p], h_sb[:])
                lgb = spool.tile([COUT, P_PAT, BS_LOCAL], F32, tag="lgb")
                nc.vector.tensor_add(
                    lgb[:], lg_ps[:],
                    b2sb[:].unsqueeze(2).broadcast_to(
                        (COUT, P_PAT, BS_LOCAL)))

                # softmax over pattern axis (temperature 0.5 -> scale 2.0)
                mx = spool.tile([COUT, BS_LOCAL], F32, tag="mx")
                nc.vector.tensor_max(mx[:], lgb[:, 0], lgb[:, 1])
                nc.vector.tensor_max(mx[:], mx[:], lgb[:, 2])
                nc.vector.tensor_max(mx[:], mx[:], lgb[:, 3])
                sd = spool.tile([COUT, P_PAT, BS_LOCAL], F32, tag="sd")
                nc.vector.tensor_sub(
                    sd[:], lgb[:],
                    mx[:].unsqueeze(1).broadcast_to((COUT, P_PAT, BS_LOCAL)))
                se = spool.tile([COUT, P_PAT, BS_LOCAL], F32, tag="se")
                nc.scalar.activation(
                    se[:], sd[:], mybir.ActivationFunctionType.Exp,
                    scale=1.0 / TEMP)
                ssum = spool.tile([COUT, BS_LOCAL], F32, tag="ssum")
                nc.vector.tensor_add(ssum[:], se[:, 0], se[:, 1])
                nc.vector.tensor_add(ssum[:], ssum[:], se[:, 2])
                nc.vector.tensor_add(ssum[:], ssum[:], se[:, 3])
                rec = spool.tile([COUT, BS_LOCAL], F32, tag="rec")
                nc.vector.reciprocal(rec[:], ssum[:])
                attn = spool.tile([COUT, P_PAT, BS_LOCAL], F32, tag="attn")
                nc.vector.tensor_mul(
                    attn[:], se[:],
                    rec[:].unsqueeze(1).broadcast_to((COUT, P_PAT, BS_LOCAL)))

            # ---- fold mask into the weight bank -------------------------
            if "mask" not in cut:
                wm = spool.tile([128, P_PAT, KK, CIN], BF16, tag="wm")
                for p in range(P_PAT):
                    for kl in range(KK):
                        nc.vector.tensor_scalar_mul(
                            wm[:, p, kl], wco[:, p, kl],
                            mb[:, p * KK + kl:p * KK + kl + 1])

            # ---- per-sample: aggregate, transpose, convolve -------------
            for s in range(BS_LOCAL):
                if "agg" in cut:
                    lhsT = lhsT_c
                else:
                    agg = aggp.tile([128, KK, CIN], BF16, tag="agg")
                    av = agg[:].rearrange("co kl ci -> co (kl ci)")
                    nc.vector.tensor_scalar_mul(
                        av, wm[:, 0].rearrange("co kl ci -> co (kl ci)"),
                        attn[:, 0, s:s + 1])
                    for p in range(1, P_PAT):
                        nc.vector.scalar_tensor_tensor(
                            av, wm[:, p].rearrange("co kl ci -> co (kl ci)"),
                            attn[:, p, s:s + 1], av,
                            op0=mybir.AluOpType.mult, op1=mybir.AluOpType.add)

                    lhsT = lhsp.tile([CIN, KK, COUT], BF16, tag="lhsT")
                    if tp_mode == "dma":
                        for kl in range(KK):
                            eng = nc.sync if kl % 2 == 0 else nc.scalar
                            eng.dma_start_transpose(lhsT[:, kl], agg[:, kl])
                    else:
                        for kl in range(KK):
                            tp_ps = ps_tp.tile([CIN, COUT], BF16, tag="tp")
                            nc.tensor.transpose(tp_ps[:], agg[:, kl], idb[:])
                            nc.scalar.copy(lhsT[:, kl], tp_ps[:])

                if "conv" in cut:
                    continue
                osb = outp.tile([COUT, H, W], BF16, tag="osb")
                for yc in range(N_CHUNK):
                    y0 = yc * YC
                    pt = ps_mm.tile([COUT, YC, W], F32, tag="pt")
                    for i, (dk, dl) in enumerate(
                            (dk, dl) for dk in range(KS) for dl in range(KS)):
                        nc.tensor.matmul(
                            pt[:],
                            lhsT[:, dk * KS + dl],
                            xs[s][:, y0 + dk:y0 + dk + YC, dl:dl + W],
                            start=(i == 0), stop=(i == KK - 1))
                    nc.scalar.copy(osb[:, y0:y0 + YC, :], pt[:])
                nc.sync.dma_start(out_d[s], osb[:])

    _split_excess_waits(nc)
    return nc


def _split_excess_waits(nc, max_inline=1):
    """Hoist extra sync waits into standalone EventSemaphore instructions.

    This walrus build rejects instructions whose encoded sync-command
    count exceeds the ISA struct capacity ("Too many sync wait
    commands") -- in practice more than one wait per compute
    instruction. Engines execute their instruction stream in order, so
    blocking on a preceding same-engine EventSemaphore is equivalent to
    the instruction carrying the wait itself.
    """
    n = 0
    for f in nc.m.functions:
        for blk in f.blocks:
            out = []
            for inst in blk.instructions:
                si = inst.sync_info
                if si is not None and len(si.on_wait) > max_inline:
                    waits = list(si.on_wait)
                    keep = waits[:max_inline]
                    for w in waits[max_inline:]:
                        n += 1
                        ev = mybir.InstEventSemaphore(
                            name=f"WSPLIT-{n}", ins=[], outs=[])
                        ev.engine = inst.engine
                        ev.sync_info = mybir.SyncInfo(on_wait=[w], on_update=[])
                        ev.debug = inst.debug
                        nc.inst_map[ev.name] = ev
                        out.append(ev)
                    inst.sync_info = mybir.SyncInfo(
                        on_wait=keep, on_update=list(si.on_update))
                out.append(inst)
            blk.instructions = out
    return n


class _Runner:
    """Cached PJRT executor for the 8-core SPMD program.

    Mirrors bass2jax.run_bass_via_pjrt's multi-core path but keeps the
    jitted shard_map callable (and the device mesh) alive across calls,
    so repeat invocations skip retracing and recompilation.
    """

    def __init__(self, nc):
        import jax
        import jax.numpy as jnp
        from jax.experimental.shard_map import shard_map
        from jax.sharding import Mesh, PartitionSpec, NamedSharding
        from concourse import bass2jax, mybir as _mb

        bass2jax.install_neuronx_cc_hook()
        self.jax = jax
        self.nc = nc
        assert nc.dbg_addr is None

        partition_name = (nc.partition_id_tensor.name
                          if nc.partition_id_tensor else None)
        in_names, out_names, out_avals, zero_shapes = [], [], [], []
        for alloc in nc.m.functions[0].allocations:
            if not isinstance(alloc, _mb.MemoryLocationSet):
                continue
            name = alloc.memorylocations[0].name
            if alloc.kind == "ExternalInput":
                if name != partition_name:
                    in_names.append(name)
            elif alloc.kind == "ExternalOutput":
                out_names.append(name)
                shape = tuple(alloc.tensor_shape)
                dtype = _mb.dt.np(alloc.dtype)
                out_avals.append(jax.core.ShapedArray(shape, dtype))
                zero_shapes.append((shape, dtype))
        self.in_names = list(in_names)
        self.out_names = out_names
        self.out_avals = out_avals
        n_params = len(in_names)
        n_outs = len(out_names)
        all_in_names = in_names + out_names
        if partition_name is not None:
            all_in_names.append(partition_name)
        donate = tuple(range(n_params, n_params + n_outs))

        def _body(*args):
            operands = list(args)
            if partition_name is not None:
                operands.append(bass2jax.partition_id_tensor())
            outs = bass2jax._bass_exec_p.bind(
                *operands,
                out_avals=tuple(out_avals),
                in_names=tuple(all_in_names),
                out_names=tuple(out_names),
                lowering_input_output_aliases=(),
                sim_require_finite=True,
                sim_require_nnan=True,
                nc=nc,
            )
            return tuple(outs)

        devices = jax.devices()[:N_CORES]
        self.mesh = Mesh(np.asarray(devices), ("core",))
        self.sharding = NamedSharding(self.mesh, PartitionSpec("core"))
        in_specs = (PartitionSpec("core"),) * (n_params + n_outs)
        out_specs = (PartitionSpec("core"),) * n_outs
        self.sharded = jax.jit(
            shard_map(_body, mesh=self.mesh, in_specs=in_specs,
                      out_specs=out_specs, check_rep=False),
            donate_argnums=donate, keep_unused=True)
        self._zero_makers = [
            jax.jit(
                (lambda sh=sh, dt=dt: jnp.zeros((N_CORES * sh[0],) + sh[1:], dt)),
                out_shardings=self.sharding)
            for sh, dt in zero_shapes
        ]

    def put_inputs(self, in_maps):
        """Concat per-core inputs on axis 0 and upload sharded."""
        cat = [
            np.concatenate([np.asarray(m[name]) for m in in_maps], axis=0)
            for name in self.in_names
        ]
        return [self.jax.device_put(a, self.sharding) for a in cat]

    def run(self, dev_inputs):
        zeros = [zm() for zm in self._zero_makers]
        outs = self.sharded(*dev_inputs, *zeros)
        self.jax.block_until_ready(outs)
        return outs

    def results(self, outs):
        res = []
        for c in range(N_CORES):
            res.append({
                name: np.asarray(outs[i]).reshape(
                    N_CORES, *self.out_avals[i].shape)[c]
                for i, name in enumerate(self.out_names)
            })
        return res


_RUNNER_CACHE = {}


def _get_runner(use_f32r=True, rep=1, mm=None):
    key = rep
    if key not in _RUNNER_CACHE:
        _RUNNER_CACHE[key] = _Runner(_get_nc(rep=rep))
    return _RUNNER_CACHE[key]


_NC_CACHE = {}


def _get_nc(use_f32r=True, rep=1, mm=None):
    key = rep
    if key not in _NC_CACHE:
        _NC_CACHE[key] = build_nc(rep=rep)
    return _NC_CACHE[key]


def make_in_maps(x, psa_w1, psa_w2, psa_b2, weight, threshold):
    x = np.asarray(x, dtype=np.float32)
    xp = np.zeros((BS, CIN, H + 2, W + 2), np.float32)
    xp[:, :, 1:H + 1, 1:W + 1] = x
    xp = xp.astype(ml_dtypes.bfloat16)
    w = np.asarray(weight, np.float32)
    wb = np.ascontiguousarray(
        w.reshape(P_PAT, COUT, CIN, KK).transpose(0, 1, 3, 2)
    ).astype(ml_dtypes.bfloat16)
    w2 = np.asarray(psa_w2, np.float32).reshape(P_PAT, COUT, HID)
    thr = np.asarray(threshold, np.float32).reshape(P_PAT, COUT)
    common = {
        "identb": np.eye(128, dtype=ml_dtypes.bfloat16),
        "wb": wb,
        "w1T": np.ascontiguousarray(np.asarray(psa_w1, np.float32).T),
        "w2T": np.ascontiguousarray(w2.transpose(2, 0, 1)),
        "b2cp": np.ascontiguousarray(
            np.asarray(psa_b2, np.float32).reshape(P_PAT, COUT).T),
        "thrm": thr.mean(axis=1, keepdims=True).T,  # [1, P]
    }
    return [
        {"x": xp[c * BS_LOCAL:(c + 1) * BS_LOCAL], **common}
        for c in range(N_CORES)
    ]


def kernel(x, psa_w1, psa_w2, psa_b2, weight, threshold):
    in_maps = make_in_maps(x, psa_w1, psa_w2, psa_b2, weight, threshold)
    try:
        r = _get_runner()
        outs = r.run(r.put_inputs(in_maps))
        res = r.results(outs)
    except Exception:
        nc = _get_nc()
        res = run_bass_kernel_spmd(nc, in_maps, list(range(N_CORES))).results
    return np.concatenate(
        [res[c]["out"] for c in range(N_CORES)], axis=0).astype(np.float32)


# revision 18
# speedup vs baseline: 34.0024x; 1.5457x over previous
"""DPPConv2d Trainium2 Bass kernel (bf16 pipeline).

Reference computation (per sample s):
  pooled = mean_{h,w} x[s]                              [Cin]
  h      = relu(pooled @ W1.T)                          [hidden]
  logits = h @ W2.T + b2                                [P*Cout]
  attn   = softmax(logits.reshape(P, Cout) / 0.5, p)    [P, Cout]
  m      = (mean_{o,i}(|W[p,:,:,k,l]| - thr[p,:]) > 0)  [P, K, K]
  agg    = sum_p attn[p, co] * m[p, kl] * W[p, co, ci, kl]
  out[s] = conv2d(x[s], agg, pad=1)                     [Cout, H, W]

Sharding: data-parallel over batch -- 8 cores x 4 samples each; the
weight bank / psa weights / threshold are replicated on every core.

Key performance facts (HW-measured):
  - bf16 matmul streams 2 moving columns/cycle (FD=512 -> ~111 ns) and
    its LDWEIGHTS is hidden by the PE reorder window; the f32r
    self-loading encoding pays the weight load inline (~178 ns).
  - So everything is bf16: x is padded+converted on the host, the
    weight bank is host-reordered to [P, Cout, k*k, Cin] bf16, conv
    accumulates in fp32 PSUM, output returns as bf16.

Per-core device pipeline:
  - SE attention entirely in [Cout, P, S] layout (free dims of 4..16,
    using stride-0 broadcast APs) -- logits via 4 tiny matmuls with
    host-pretransposed W2; no attention transposes needed.
  - binary spatial mask on device (|W| reduce, ones-matmul partition
    sums, is_gt, ones-matmul broadcast), folded into the bank (wm).
  - per-sample aggregated weights via tensor_scalar/stt chains in
    [co, kl*ci] layout with attn as per-partition scalar.
  - lhsT [ci, kl, co] built with 9 SBUF->SBUF DMA transposes (XBAR),
    zero PE/scalar cost.
  - conv as 9 shifted accumulating matmuls per 8-row output chunk.
"""

import os
import sys

try:
    import concourse.bass as bass  # noqa: F401
except Exception:  # pragma: no cover
    sys.path.insert(0, "/opt/trn_rl_repo")

from contextlib import ExitStack

import numpy as np
import ml_dtypes

import concourse.bass as bass
import concourse.tile as tile
from concourse import mybir
from concourse.bass_utils import run_bass_kernel_spmd

N_CORES = 8
BS = 32
BS_LOCAL = BS // N_CORES  # 4
CIN = 128
COUT = 128
H = W = 64
P_PAT = 4
KS = 3
KK = KS * KS
HID = 33
TEMP = 0.5
YC = 8          # output rows per conv chunk
N_CHUNK = H // YC

F32 = mybir.dt.float32
BF16 = mybir.dt.bfloat16


def build_nc(rep=1, tp_mode=None):
    if tp_mode is None:
        tp_mode = os.environ.get("DPP_TP", "dma")
    cut = set(os.environ.get("DPP_CUT", "").split(","))  # timing bisect only
    agg_f32 = os.environ.get("DPP_AGG", "f32") == "f32"
    wdt = F32 if agg_f32 else BF16
    nc = bass.Bass("TRN2", target_bir_lowering=False, debug=False,
                   num_swdge_queues=4)

    x_d = nc.dram_tensor("x", [BS_LOCAL, CIN, H + 2, W + 2], BF16,
                         kind="ExternalInput")
    idb_d = nc.dram_tensor("identb", [128, 128], BF16, kind="ExternalInput")
    wb_d = nc.dram_tensor("wb", [P_PAT, COUT, KK, CIN], BF16,
                          kind="ExternalInput")
    wbf_d = nc.dram_tensor("wbf", [P_PAT, COUT, KK, CIN], F32,
                           kind="ExternalInput")
    w1_d = nc.dram_tensor("w1T", [CIN, HID], F32, kind="ExternalInput")
    w2_d = nc.dram_tensor("w2T", [HID, P_PAT, COUT], F32, kind="ExternalInput")
    b2_d = nc.dram_tensor("b2cp", [COUT, P_PAT], F32, kind="ExternalInput")
    th_d = nc.dram_tensor("thrm", [1, P_PAT], F32, kind="ExternalInput")
    out_d = nc.dram_tensor("out", [BS_LOCAL, COUT, H, W], BF16,
                           kind="ExternalOutput")

    with tile.TileContext(nc) as tc, ExitStack() as ctx:
        consts = ctx.enter_context(tc.tile_pool(name="consts", bufs=1))
        spool = ctx.enter_context(tc.tile_pool(name="spool", bufs=2))
        xpool = ctx.enter_context(tc.tile_pool(name="xpool", bufs=2))
        wpool = ctx.enter_context(tc.tile_pool(name="wpool", bufs=2))
        aggp = ctx.enter_context(tc.tile_pool(name="aggp", bufs=2))
        lhsp = ctx.enter_context(tc.tile_pool(name="lhsp", bufs=3))
        outp = ctx.enter_context(tc.tile_pool(name="outp", bufs=2))
        ps_small = ctx.enter_context(
            tc.tile_pool(name="ps_small", bufs=2, space="PSUM"))
        ps_mm = ctx.enter_context(
            tc.tile_pool(name="ps_mm", bufs=4, space="PSUM"))
        if tp_mode == "pe":
            ps_tp = ctx.enter_context(
                tc.tile_pool(name="ps_tp", bufs=2, space="PSUM"))

        ones_col = consts.tile([128, 1], F32, tag="ones_col")
        nc.vector.memset(ones_col[:], 1.0)
        ones_row = consts.tile([1, 128], F32, tag="ones_row")
        nc.vector.memset(ones_row[:], 1.0)
        idb = consts.tile([128, 128], BF16, tag="idb")
        nc.sync.dma_start(idb[:], idb_d[:])
        if "agg" in cut:
            lhsT_c = consts.tile([CIN, KK, COUT], BF16, tag="lhsT_c")
            nc.vector.memset(lhsT_c[:].bitcast(F32), 0.5)
        if "se" in cut:
            attn_c = consts.tile([COUT, P_PAT, BS_LOCAL], F32, tag="attn_c")
            nc.vector.memset(attn_c[:], 0.25)

        for _rep in range(rep):
            # ---- input DMAs ---------------------------------------------
            xs = [xpool.tile([CIN, H + 2, W + 2], BF16, tag=f"xs{s}",
                             name=f"xs{s}") for s in range(BS_LOCAL)]
            for s in range(BS_LOCAL):
                nc.gpsimd.dma_start(xs[s][:], x_d[s])
            wco = wpool.tile([128, P_PAT, KK, CIN], wdt, tag="wco")
            nc.sync.dma_start(
                wco[:], (wbf_d if agg_f32 else wb_d)[:].rearrange(
                    "p co kl ci -> co p kl ci"))
            w1sb = spool.tile([CIN, HID], F32, tag="w1sb")
            nc.sync.dma_start(w1sb[:], w1_d[:])
            w2sb = spool.tile([HID, P_PAT, COUT], F32, tag="w2sb")
            nc.sync.dma_start(w2sb[:], w2_d[:])
            b2sb = spool.tile([COUT, P_PAT], F32, tag="b2sb")
            nc.sync.dma_start(b2sb[:], b2_d[:])
            thsb = spool.tile([1, P_PAT], F32, tag="thsb")
            nc.sync.dma_start(thsb[:], th_d[:])

            # ---- pooled sums (mean folded into the relu scale) ----------
            pooled = spool.tile([CIN, BS_LOCAL], F32, tag="pooled")
            if "se" not in cut:
                for s in range(BS_LOCAL):
                    nc.vector.reduce_sum(
                        pooled[:, s:s + 1], xs[s][:],
                        axis=mybir.AxisListType.XY)

            # ---- binary spatial mask ------------------------------------
            wsum = spool.tile([128, P_PAT * KK], F32, tag="wsum")
            if "mask" in cut:
                wm = wco
            else:
                nc.vector.reduce_sum(
                    wsum[:].rearrange("co (p kl) -> co p kl", p=P_PAT),
                    wco[:], axis=mybir.AxisListType.X,
                    apply_absolute_value=True)
                ws_ps = ps_small.tile([1, P_PAT * KK], F32, tag="sm")
                nc.tensor.matmul(ws_ps[:], ones_col[:], wsum[:])
                z_row = spool.tile([1, P_PAT * KK], F32, tag="z_row")
                for p in range(P_PAT):
                    nc.vector.tensor_scalar(
                        z_row[0:1, p * KK:(p + 1) * KK],
                        ws_ps[0:1, p * KK:(p + 1) * KK],
                        1.0 / (COUT * CIN), thsb[0:1, p:p + 1],
                        op0=mybir.AluOpType.mult,
                        op1=mybir.AluOpType.subtract)
                m_row = spool.tile([1, P_PAT * KK], F32, tag="m_row")
                nc.vector.tensor_scalar(
                    m_row[:], z_row[:], 0.0, None, op0=mybir.AluOpType.is_gt)
                mb_ps = ps_small.tile([128, P_PAT * KK], F32, tag="sm")
                nc.tensor.matmul(mb_ps[:], ones_row[:], m_row[:])
                mb = spool.tile([128, P_PAT * KK], F32, tag="mb")
                nc.scalar.copy(mb[:], mb_ps[:])

            # ---- SE attention MLP in [co, p, s] layout ------------------
            if "se" in cut:
                attn = attn_c
            else:
                h_ps = ps_small.tile([HID, BS_LOCAL], F32, tag="sm")
                nc.tensor.matmul(h_ps[:], w1sb[:], pooled[:])
                h_sb = spool.tile([HID, BS_LOCAL], F32, tag="h_sb")
                nc.scalar.activation(
                    h_sb[:], h_ps[:], mybir.ActivationFunctionType.Relu,
                    scale=1.0 / (H * W))
                lg_ps = ps_small.tile([COUT, P_PAT, BS_LOCAL], F32, tag="sm")
                for p in range(P_PAT):
                    nc.tensor.matmul(lg_ps[:, p], w2sb[:, p], h_sb[:])
                lgb = spool.tile([COUT, P_PAT, BS_LOCAL], F32, tag="lgb")
                nc.vector.tensor_add(
                    lgb[:], lg_ps[:],
                    b2sb[:].unsqueeze(2).broadcast_to(
                        (COUT, P_PAT, BS_LOCAL)))

                # softmax over pattern axis (temperature 0.5 -> scale 2.0)
                mx = spool.tile([COUT, BS_LOCAL], F32, tag="mx")
                nc.vector.tensor_max(mx[:], lgb[:, 0], lgb[:, 1])
                nc.vector.tensor_max(mx[:], mx[:], lgb[:, 2])
                nc.vector.tensor_max(mx[:], mx[:], lgb[:, 3])
                sd = spool.tile([COUT, P_PAT, BS_LOCAL], F32, tag="sd")
                nc.vector.tensor_sub(
                    sd[:], lgb[:],
                    mx[:].unsqueeze(1).broadcast_to((COUT, P_PAT, BS_LOCAL)))
                se = spool.tile([COUT, P_PAT, BS_LOCAL], F32, tag="se")
                nc.scalar.activation(
                    se[:], sd[:], mybir.ActivationFunctionType.Exp,
                    scale=1.0 / TEMP)
                ssum = spool.tile([COUT, BS_LOCAL], F32, tag="ssum")
                nc.vector.tensor_add(ssum[:], se[:, 0], se[:, 1])
                nc.vector.tensor_add(ssum[:], ssum[:], se[:, 2])
                nc.vector.tensor_add(ssum[:], ssum[:], se[:, 3])
                rec = spool.tile([COUT, BS_LOCAL], F32, tag="rec")
                nc.vector.reciprocal(rec[:], ssum[:])
                attn = spool.tile([COUT, P_PAT, BS_LOCAL], F32, tag="attn")
                nc.vector.tensor_mul(
                    attn[:], se[:],
                    rec[:].unsqueeze(1).broadcast_to((COUT, P_PAT, BS_LOCAL)))

            # ---- fold mask into the weight bank -------------------------
            if "mask" not in cut:
                wm = spool.tile([128, P_PAT, KK, CIN], wdt, tag="wm")
                for p in range(P_PAT):
                    for kl in range(KK):
                        nc.vector.tensor_scalar_mul(
                            wm[:, p, kl], wco[:, p, kl],
                            mb[:, p * KK + kl:p * KK + kl + 1])

            # ---- per-sample: aggregate, transpose, convolve -------------
            for s in range(BS_LOCAL):
                if "agg" in cut:
                    lhsT = lhsT_c
                else:
                    agg = aggp.tile([128, KK, CIN], wdt, tag="agg")
                    av = agg[:].rearrange("co kl ci -> co (kl ci)")
                    nc.vector.tensor_scalar_mul(
                        av, wm[:, 0].rearrange("co kl ci -> co (kl ci)"),
                        attn[:, 0, s:s + 1])
                    for p in range(1, P_PAT):
                        nc.vector.scalar_tensor_tensor(
                            av, wm[:, p].rearrange("co kl ci -> co (kl ci)"),
                            attn[:, p, s:s + 1], av,
                            op0=mybir.AluOpType.mult, op1=mybir.AluOpType.add)
                    if agg_f32:
                        aggb = aggp.tile([128, KK, CIN], BF16, tag="aggb")
                        nc.scalar.copy(
                            aggb[:].rearrange("co kl ci -> co (kl ci)"), av)
                        agg = aggb

                    lhsT = lhsp.tile([CIN, KK, COUT], BF16, tag="lhsT")
                    if tp_mode == "dma":
                        for kl in range(KK):
                            eng = nc.sync if kl % 2 == 0 else nc.scalar
                            eng.dma_start_transpose(lhsT[:, kl], agg[:, kl])
                    else:
                        for kl in range(KK):
                            tp_ps = ps_tp.tile([CIN, COUT], BF16, tag="tp")
                            nc.tensor.transpose(tp_ps[:], agg[:, kl], idb[:])
                            nc.scalar.copy(lhsT[:, kl], tp_ps[:])

                if "conv" in cut:
                    continue
                osb = outp.tile([COUT, H, W], BF16, tag="osb")
                for yc in range(N_CHUNK):
                    y0 = yc * YC
                    pt = ps_mm.tile([COUT, YC, W], F32, tag="pt")
                    for i, (dk, dl) in enumerate(
                            (dk, dl) for dk in range(KS) for dl in range(KS)):
                        nc.tensor.matmul(
                            pt[:],
                            lhsT[:, dk * KS + dl],
                            xs[s][:, y0 + dk:y0 + dk + YC, dl:dl + W],
                            start=(i == 0), stop=(i == KK - 1))
                    nc.scalar.copy(osb[:, y0:y0 + YC, :], pt[:])
                nc.sync.dma_start(out_d[s], osb[:])

    _split_excess_waits(nc)
    return nc


def _split_excess_waits(nc, max_inline=1):
    """Hoist extra sync waits into standalone EventSemaphore instructions.

    This walrus build rejects instructions whose encoded sync-command
    count exceeds the ISA struct capacity ("Too many sync wait
    commands") -- in practice more than one wait per compute
    instruction. Engines execute their instruction stream in order, so
    blocking on a preceding same-engine EventSemaphore is equivalent to
    the instruction carrying the wait itself.
    """
    n = 0
    for f in nc.m.functions:
        for blk in f.blocks:
            out = []
            for inst in blk.instructions:
                si = inst.sync_info
                if si is not None and len(si.on_wait) > max_inline:
                    waits = list(si.on_wait)
                    keep = waits[:max_inline]
                    for w in waits[max_inline:]:
                        n += 1
                        ev = mybir.InstEventSemaphore(
                            name=f"WSPLIT-{n}", ins=[], outs=[])
                        ev.engine = inst.engine
                        ev.sync_info = mybir.SyncInfo(on_wait=[w], on_update=[])
                        ev.debug = inst.debug
                        nc.inst_map[ev.name] = ev
                        out.append(ev)
                    inst.sync_info = mybir.SyncInfo(
                        on_wait=keep, on_update=list(si.on_update))
                out.append(inst)
            blk.instructions = out
    return n


class _Runner:
    """Cached PJRT executor for the 8-core SPMD program.

    Mirrors bass2jax.run_bass_via_pjrt's multi-core path but keeps the
    jitted shard_map callable (and the device mesh) alive across calls,
    so repeat invocations skip retracing and recompilation.
    """

    def __init__(self, nc):
        import jax
        import jax.numpy as jnp
        from jax.experimental.shard_map import shard_map
        from jax.sharding import Mesh, PartitionSpec, NamedSharding
        from concourse import bass2jax, mybir as _mb

        bass2jax.install_neuronx_cc_hook()
        self.jax = jax
        self.nc = nc
        assert nc.dbg_addr is None

        partition_name = (nc.partition_id_tensor.name
                          if nc.partition_id_tensor else None)
        in_names, out_names, out_avals, zero_shapes = [], [], [], []
        for alloc in nc.m.functions[0].allocations:
            if not isinstance(alloc, _mb.MemoryLocationSet):
                continue
            name = alloc.memorylocations[0].name
            if alloc.kind == "ExternalInput":
                if name != partition_name:
                    in_names.append(name)
            elif alloc.kind == "ExternalOutput":
                out_names.append(name)
                shape = tuple(alloc.tensor_shape)
                dtype = _mb.dt.np(alloc.dtype)
                out_avals.append(jax.core.ShapedArray(shape, dtype))
                zero_shapes.append((shape, dtype))
        self.in_names = list(in_names)
        self.out_names = out_names
        self.out_avals = out_avals
        n_params = len(in_names)
        n_outs = len(out_names)
        all_in_names = in_names + out_names
        if partition_name is not None:
            all_in_names.append(partition_name)
        donate = tuple(range(n_params, n_params + n_outs))

        def _body(*args):
            operands = list(args)
            if partition_name is not None:
                operands.append(bass2jax.partition_id_tensor())
            outs = bass2jax._bass_exec_p.bind(
                *operands,
                out_avals=tuple(out_avals),
                in_names=tuple(all_in_names),
                out_names=tuple(out_names),
                lowering_input_output_aliases=(),
                sim_require_finite=True,
                sim_require_nnan=True,
                nc=nc,
            )
            return tuple(outs)

        devices = jax.devices()[:N_CORES]
        self.mesh = Mesh(np.asarray(devices), ("core",))
        self.sharding = NamedSharding(self.mesh, PartitionSpec("core"))
        in_specs = (PartitionSpec("core"),) * (n_params + n_outs)
        out_specs = (PartitionSpec("core"),) * n_outs
        self.sharded = jax.jit(
            shard_map(_body, mesh=self.mesh, in_specs=in_specs,
                      out_specs=out_specs, check_rep=False),
            donate_argnums=donate, keep_unused=True)
        self._zero_makers = [
            jax.jit(
                (lambda sh=sh, dt=dt: jnp.zeros((N_CORES * sh[0],) + sh[1:], dt)),
                out_shardings=self.sharding)
            for sh, dt in zero_shapes
        ]

    def put_inputs(self, in_maps):
        """Concat per-core inputs on axis 0 and upload sharded."""
        cat = [
            np.concatenate([np.asarray(m[name]) for m in in_maps], axis=0)
            for name in self.in_names
        ]
        return [self.jax.device_put(a, self.sharding) for a in cat]

    def run(self, dev_inputs):
        zeros = [zm() for zm in self._zero_makers]
        outs = self.sharded(*dev_inputs, *zeros)
        self.jax.block_until_ready(outs)
        return outs

    def results(self, outs):
        res = []
        for c in range(N_CORES):
            res.append({
                name: np.asarray(outs[i]).reshape(
                    N_CORES, *self.out_avals[i].shape)[c]
                for i, name in enumerate(self.out_names)
            })
        return res


_RUNNER_CACHE = {}


def _get_runner(use_f32r=True, rep=1, mm=None):
    key = rep
    if key not in _RUNNER_CACHE:
        _RUNNER_CACHE[key] = _Runner(_get_nc(rep=rep))
    return _RUNNER_CACHE[key]


_NC_CACHE = {}


def _get_nc(use_f32r=True, rep=1, mm=None):
    key = rep
    if key not in _NC_CACHE:
        _NC_CACHE[key] = build_nc(rep=rep)
    return _NC_CACHE[key]


def make_in_maps(x, psa_w1, psa_w2, psa_b2, weight, threshold):
    x = np.asarray(x, dtype=np.float32)
    xp = np.zeros((BS, CIN, H + 2, W + 2), np.float32)
    xp[:, :, 1:H + 1, 1:W + 1] = x
    xp = xp.astype(ml_dtypes.bfloat16)
    w = np.asarray(weight, np.float32)
    wb = np.ascontiguousarray(
        w.reshape(P_PAT, COUT, CIN, KK).transpose(0, 1, 3, 2)
    ).astype(ml_dtypes.bfloat16)
    w2 = np.asarray(psa_w2, np.float32).reshape(P_PAT, COUT, HID)
    thr = np.asarray(threshold, np.float32).reshape(P_PAT, COUT)
    wbf = np.ascontiguousarray(
        w.reshape(P_PAT, COUT, CIN, KK).transpose(0, 1, 3, 2))
    common = {
        "identb": np.eye(128, dtype=ml_dtypes.bfloat16),
        "wb": wb,
        "wbf": wbf,
        "w1T": np.ascontiguousarray(np.asarray(psa_w1, np.float32).T),
        "w2T": np.ascontiguousarray(w2.transpose(2, 0, 1)),
        "b2cp": np.ascontiguousarray(
            np.asarray(psa_b2, np.float32).reshape(P_PAT, COUT).T),
        "thrm": thr.mean(axis=1, keepdims=True).T,  # [1, P]
    }
    return [
        {"x": xp[c * BS_LOCAL:(c + 1) * BS_LOCAL], **common}
        for c in range(N_CORES)
    ]


def kernel(x, psa_w1, psa_w2, psa_b2, weight, threshold):
    in_maps = make_in_maps(x, psa_w1, psa_w2, psa_b2, weight, threshold)
    try:
        r = _get_runner()
        outs = r.run(r.put_inputs(in_maps))
        res = r.results(outs)
    except Exception:
        nc = _get_nc()
        res = run_bass_kernel_spmd(nc, in_maps, list(range(N_CORES))).results
    return np.concatenate(
        [res[c]["out"] for c in range(N_CORES)], axis=0).astype(np.float32)
